# revision 4
# baseline (speedup 1.0000x reference)
"""BDGCN (dual-diffusion graph conv) Trainium2 kernel, v2.

Math (per batch b):
  m1[k,m,c,l] = sum_n X[n,c,l] G[k,n,m]
  m2[m,d,k,j,l] = sum_c m1[k,m,c,l] G[j,c,d]
  out[m,d,h] = relu(sum_{k,j,l} m2[m,d,k,j,l] W[k*96+j*32+l, h] + b[h])

Sharding: data-parallel over batch; B=8 -> one batch per NeuronCore,
G/W/b replicated. No collectives.

v2 design (all matmul operands bf16, fp32 psum accumulation):
  Phase 1 (contract n): lhsT = X[n, c-chunk @ fixed l], rhs packs two
    G_k side by side -> psum [c128, 512] (+ a 256-wide for k=2), accum
    over 2 n-chunks.  Copies write m1[k][cc] SBUF bf16 tiles with free
    layout (g64, l32, r4), m = 4g + r (full m range, no m-half split).
  Phase 2 (contract c): lhsT = m1 column block g, rhs packs G_j0|G_j1
    -> psum [ (l,r)128, 512 ] per (g, k), + 256-wide for j=2; accum
    over 2 c-chunks; copy to bf16 m2 tiles.
  Phase 3 (contract (k,j,l)): lhsT = m2[:, d-chunk], rhs =
    block-diagonal W [(l,r)128, (r,h)256], 9 accumulating matmuls plus
    a bias matmul (ones[1,128] x brow[1,256]), then relu fused into the
    PSUM->SBUF copy, DMA out to [m, d, h].

PSUM->SBUF copies round-robin across DVE / Activation / Pool engines
(DVE alone was the bottleneck: ~375us busy in the v1 cost-model sim).

Walrus-build workarounds baked in: Tile's exit drain is split into
single-wait drains (_patch_tile_drain) and any instruction carrying >1
semaphore wait gets extra waits hoisted onto NoOps (_split_multi_waits).
"""

import numpy as np

B, N, L, K, H = 8, 256, 32, 3, 64
P = 128  # partitions

_CACHE = {}


def _patch_tile_drain():
    """This container's walrus build rejects instructions carrying more
    than one semaphore wait; Tile's exit emits one drain with N waits.
    Split it into N single-wait drains."""
    import concourse.mybir as mybir
    import concourse.tile as tile

    if getattr(tile.TileContext, "_drain_split_patched", False):
        return

    def patched(self, tick_clock, wait_clock):
        from concourse.vector_clock import ScopedClock

        nc = self.nc
        probe = nc.sync.drain()
        wait_clock.add_sem_waits(
            probe.ins, ScopedClock({None: tick_clock.global_clock})
        )
        si = probe.ins.sync_info
        waits = list(si.on_wait) if si is not None else []
        if len(waits) > 1:
            si.on_wait = [waits[0]]
            for w in waits[1:]:
                d = nc.sync.drain()
                d.ins.sync_info = mybir.SyncInfo(on_update=[], on_wait=[w])
        nc.all_engine_barrier()
        assert self.sems is not None
        popped = nc._tile_sem_poison_stack.pop()
        assert popped is self._sem_poison
        nc.clear_and_free_semaphores(list(self.sems.allocated().values()))
        nc.all_engine_barrier()

    tile.TileContext._drain_and_barrier = patched
    tile.TileContext._drain_split_patched = True


def _build_nc(reps=1):
    import concourse.bass as bass
    import concourse.mybir as mybir
    import concourse.tile as tile

    _patch_tile_drain()

    f32 = mybir.dt.float32
    bf16 = mybir.dt.bfloat16
    relu = mybir.ActivationFunctionType.Relu
    nc = bass.Bass("TRN2", target_bir_lowering=False, debug=False)

    Xd = nc.dram_tensor("X", [N, N, L], bf16, kind="ExternalInput")
    GBd = nc.dram_tensor("GB", [K, N, N], bf16, kind="ExternalInput")
    Wr = nc.dram_tensor("WR", [K * K, P, 4 * H], bf16, kind="ExternalInput")
    Brd = nc.dram_tensor("BR", [1, 4 * H], bf16, kind="ExternalInput")
    Od = nc.dram_tensor("OUT", [N, N, H], f32, kind="ExternalOutput")

    NC2 = N // P  # 2 chunks of 128 along n or c
    MG = 4        # m's per group in phase 2/3
    NG = N // MG  # 64 groups over full m

    # Round-robin engine assignment for PSUM->SBUF copies.  Each entry:
    # (copy_fn, relu_copy_fn) factories bound to an engine.
    rr_state = [0]

    # GPSIMD cannot access PSUM on this toolchain, so PSUM->SBUF work
    # round-robins across DVE and Activation only.
    def _copy(out, in_):
        e = rr_state[0] % 2
        rr_state[0] += 1
        if e == 0:
            nc.vector.tensor_copy(out, in_)
        else:
            nc.scalar.copy(out, in_)

    def _relu_copy(out, in_):
        e = rr_state[0] % 2
        rr_state[0] += 1
        if e == 0:
            nc.vector.tensor_scalar_max(out, in_, 0.0)
        else:
            nc.scalar.activation(out, in_, relu)

    with tile.TileContext(nc) as tc:
        with (
            tc.tile_pool(name="big", bufs=1) as big,
            tc.tile_pool(name="m2p", bufs=6) as m2p,
            tc.tile_pool(name="outp", bufs=4) as outp,
            tc.tile_pool(name="ps1", bufs=2, space="PSUM") as ps1p,
            tc.tile_pool(name="ps2", bufs=2, space="PSUM") as ps2p,
            tc.tile_pool(name="ps3", bufs=2, space="PSUM") as ps3p,
        ):
            # ---- resident loads ----
            xsb = big.tile([P, NC2 * N * L], bf16, tag="xsb")
            x4 = xsb.rearrange("p (b c l) -> p b c l", b=NC2, c=N)
            nc.sync.dma_start(
                out=x4, in_=Xd[:, :, :].rearrange("(b p) c l -> p b c l", p=P)
            )
            gbsb = big.tile([P, K * NC2 * N], bf16, tag="gbsb")
            gb4 = gbsb.rearrange("p (k b m) -> p k b m", k=K, b=NC2)
            nc.sync.dma_start(
                out=gb4,
                in_=GBd[:, :, :].rearrange("k (b p) m -> p k b m", p=P),
            )
            wsb = big.tile([P, K * K * MG * H], bf16, tag="wsb")
            w3 = wsb.rearrange("p (q c) -> p q c", q=K * K)
            nc.sync.dma_start(
                out=w3, in_=Wr[:, :, :].rearrange("q p c -> p q c")
            )
            brow = big.tile([1, MG * H], bf16, tag="brow")
            nc.sync.dma_start(out=brow, in_=Brd[:, :])
            ones = big.tile([1, P], bf16, tag="ones")
            nc.vector.memset(ones, 1.0)

            m1 = {}
            for k in range(K):
                for cc in range(NC2):
                    m1[k, cc] = big.tile(
                        [P, NG * L * MG], bf16, tag=f"m1_{k}_{cc}",
                        name=f"m1_{k}_{cc}",
                    )

            for _rep in range(reps):
                # ---- phase 1: m1[k][cc] over full m, k packed 2+1 ----
                for cc in range(NC2):
                    for l in range(L):
                        # k = 0,1 packed into one 512-wide psum
                        ps = ps1p.tile([P, 2 * N], f32, tag="ps1a")
                        for nchk in range(NC2):
                            nc.tensor.matmul(
                                ps,
                                lhsT=x4[:, nchk, cc * P:(cc + 1) * P, l],
                                rhs=gb4[:, 0:2, nchk, :],
                                start=(nchk == 0),
                                stop=(nchk == NC2 - 1),
                            )
                        psb = ps.rearrange("p (k m) -> p k m", k=2)
                        # k = 2 alone in a 256-wide psum (borrows the ps3
                        # pool, which is idle during phase 1)
                        ps_c = ps3p.tile([P, N], f32, tag="ps3")
                        for nchk in range(NC2):
                            nc.tensor.matmul(
                                ps_c,
                                lhsT=x4[:, nchk, cc * P:(cc + 1) * P, l],
                                rhs=gb4[:, 2, nchk, :],
                                start=(nchk == 0),
                                stop=(nchk == NC2 - 1),
                            )
                        for k in range(K):
                            m1w = m1[k, cc].rearrange(
                                "p (g l r) -> p g l r", g=NG, l=L
                            )
                            src = psb[:, k, :] if k < 2 else ps_c
                            srcv = src.rearrange("p (g r) -> p g r", g=NG)
                            _copy(m1w[:, :, l, :], srcv)

                # ---- phases 2 + 3, per group of 4 m's ----
                for g in range(NG):
                    m2sb = {}  # (k, j) -> (tile, col offset)
                    for k in range(K):
                        # j = 0,1 packed; j = 2 alone
                        ps2 = ps2p.tile([P, 2 * N], f32, tag="ps2a")
                        for cc in range(NC2):
                            lv = m1[k, cc][:, g * P:(g + 1) * P]
                            nc.tensor.matmul(
                                ps2,
                                lhsT=lv,
                                rhs=gb4[:, 0:2, cc, :],
                                start=(cc == 0),
                                stop=(cc == NC2 - 1),
                            )
                        ps2c = ps2p.tile([P, N], f32, tag="ps2b")
                        for cc in range(NC2):
                            lv = m1[k, cc][:, g * P:(g + 1) * P]
                            nc.tensor.matmul(
                                ps2c,
                                lhsT=lv,
                                rhs=gb4[:, 2, cc, :],
                                start=(cc == 0),
                                stop=(cc == NC2 - 1),
                            )
                        ta = m2p.tile([P, 2 * N], bf16, tag="m2a")
                        _copy(ta, ps2)
                        tb = m2p.tile([P, N], bf16, tag="m2b")
                        _copy(tb, ps2c)
                        m2sb[k, 0] = (ta, 0)
                        m2sb[k, 1] = (ta, N)
                        m2sb[k, 2] = (tb, 0)
                    for dc in range(NC2):
                        ps3 = ps3p.tile([P, MG * H], f32, tag="ps3")
                        for idx in range(K * K):
                            k, j = divmod(idx, K)
                            t, off = m2sb[k, j]
                            nc.tensor.matmul(
                                ps3,
                                lhsT=t[:, off + dc * P:off + (dc + 1) * P],
                                rhs=w3[:, idx, :],
                                start=(idx == 0),
                                stop=False,
                            )
                        nc.tensor.matmul(
                            ps3, lhsT=ones, rhs=brow, start=False, stop=True
                        )
                        ost = outp.tile([P, MG * H], f32, tag="ost")
                        _relu_copy(ost, ps3)
                        mbase = g * MG
                        dst = Od[mbase:mbase + MG, dc * P:(dc + 1) * P, :]
                        nc.sync.dma_start(
                            out=dst.rearrange("m d h -> d m h"), in_=ost
                        )
    _split_multi_waits(nc)
    return nc


def _split_multi_waits(nc):
    """This walrus build accepts at most one semaphore wait per
    instruction; Tile emits up to ~2-4.  Hoist extra waits onto NoOp
    instructions inserted just before, on the same engine."""
    import concourse.mybir as mybir

    n_split = 0
    for fn in nc.m.functions:
        for bb in fn.blocks:
            insts = bb.instructions
            new = []
            for inst in insts:
                si = inst.sync_info
                waits = list(si.on_wait) if si is not None else []
                if len(waits) > 1:
                    for w in waits[:-1]:
                        nop = mybir.InstNoOp(
                            name=nc.get_next_instruction_name(), ins=[], outs=[]
                        )
                        nop.engine = inst.engine
                        nop.sync_info = mybir.SyncInfo(
                            on_update=[], on_wait=[w]
                        )
                        new.append(nop)
                        n_split += 1
                    si.on_wait = [waits[-1]]
                new.append(inst)
            if n_split:
                bb.instructions = new
    return n_split


def _get_nc():
    if "nc" not in _CACHE:
        _CACHE["nc"] = _build_nc()
    return _CACHE["nc"]


def _prep(G, W, b):
    # Block-diagonal W for phase 3: rows indexed (l, r) with r = m-within-
    # group, cols (r'', h); nonzero only when r == r''.
    import ml_dtypes

    MG = 4
    Wbd = np.zeros((K * K, P, MG * H), dtype=np.float32)
    for k in range(K):
        for j in range(K):
            blk = W[k * (K * L) + j * L:k * (K * L) + (j + 1) * L, :]  # [L, H]
            for l in range(L):
                for r in range(MG):
                    Wbd[k * K + j, l * MG + r, r * H:(r + 1) * H] = blk[l]
    Br = np.tile(b[None, :], (1, MG))
    return (
        np.ascontiguousarray(Wbd).astype(ml_dtypes.bfloat16),
        Br.astype(ml_dtypes.bfloat16),
    )


def kernel(X, G, W, b):
    import ml_dtypes
    from concourse.bass_utils import run_bass_kernel_spmd

    X = np.ascontiguousarray(X, dtype=np.float32)
    G = np.ascontiguousarray(G, dtype=np.float32)
    W = np.ascontiguousarray(W, dtype=np.float32)
    b = np.ascontiguousarray(b, dtype=np.float32)
    nc = _get_nc()
    Wr, Br = _prep(G, W, b)
    Xb = X.astype(ml_dtypes.bfloat16)
    Gb = G.astype(ml_dtypes.bfloat16)
    in_maps = [
        {"X": Xb[i], "GB": Gb, "WR": Wr, "BR": Br} for i in range(B)
    ]
    res = run_bass_kernel_spmd(nc, in_maps, list(range(B)))
    out = np.stack([res.results[i]["OUT"] for i in range(B)], axis=0)
    return out
